# revision 8
# baseline (speedup 1.0000x reference)
"""3-layer GCN on 8 Trainium2 NeuronCores (Bass/Tile), batched-gather version.

Math: with A = D^-1/2 (Adj + I) D^-1/2 (PyG GCNConv norm, self-loops),
each layer is h' = leaky_relu(A h W + b). Factor A h = dinv * ((Adj+I)(dinv*h)),
so aggregation is an unweighted gather-sum over in-edges (self-loop folded in
as an ordinary edge) of the row-scaled table u = dinv*(h@W).

Device strategy (per core, nodes sharded 6250/core):
- The per-layer table u is bf16 [rows, 128] (feature dim padded 96->128 so a
  row is 256 B, the minimum dma_gather elem size). Node positions inside each
  shard are split into a "lo" group (4096 positions, blocks 0..31) and a "hi"
  group (2176 positions, blocks 32..48) so each AllGather'd table region
  (8*4096=32768 / 8*2176=17408 rows) stays within the signed-int16 index range
  of the Q7 dma_gather ucode.
- Per layer: two AllGathers (lo region first) replicate the table, then
  dma_gather instructions (non-transpose -> row-major [128, cols, 128] wide
  tiles; idx i lands at [i%128, i//128, :]) pull all neighbor rows using
  host-built slot-major index lists. Each instruction moves <=1024 rows (the
  SWDGE descriptor-ring capacity - found empirically; bigger wedges the
  device) but that still amortizes the ~1us per-instruction SWDGE overhead
  ~8x better than per-column indirect DMA.
- Per target block: pairwise tree-sum (bf16 -> fp32) over gathered columns
  ([:, :, :96] slices), scale by dinv[target], add bias, Lrelu on the scalar
  engine, PE-transpose, then the next layer's matmul.
- Padding: position 4095 and positions 6251..6271 of each shard are reserved
  zero rows (dinv=0 there forces u rows to 0); grid padding points at them.
- Host-side: targets are assigned to blocks by a window-LPT heuristic that
  first equalizes per-block lo-degree, then balances hi-degree inside small
  windows, minimizing gather padding (~1.18x over the dense lower bound).
- Index lists are int16, wrapped [16, n/16] (idx i at [i%16, i//16]) and
  replicated to all 128 partitions (the Q7 pairs read (queue+1)*2*16 SIMD
  channels).
"""
import os
import numpy as np
import ml_dtypes
from contextlib import ExitStack

import concourse.bass as bass
import concourse.tile as tile
from concourse import bacc, mybir
from concourse.bass_utils import run_bass_kernel_spmd
from concourse.masks import make_identity

N = 50000
E = 800000
IN_F = 128
H = 96
HP = 128                  # padded feature dim (table row = 256B bf16)
C_OUT = 21
CORES = 8
SH = N // CORES           # 6250 real nodes per core
NBLK = 49
SHP = NBLK * 128          # 6272 padded positions per shard
LOB = 32                  # lo blocks per shard
HIB = NBLK - LOB          # 17 hi blocks
LOP = LOB * 128           # 4096 lo positions (4095 real + 1 zero)
HIP = HIB * 128           # 2176 hi positions (2155 real + 21 zero)
LO_TBL = CORES * LOP      # 32768 (= int16 max + 1: just fits)
HI_TBL = CORES * HIP      # 17408
LCH = LOP // 2            # lo AllGather chunk: 2048 positions (16 blocks)
LO_ZERO = CORES * LCH + 2047  # zero row (pos 4095, shard 0) in lo table
HI_ZERO = SH - (LOP - 1)  # 2155: shard-0 zero row in hi table
GMAX = 8                  # max 8 columns (1024 idxs) per dma_gather
SLOPE = 0.01

F32 = mybir.dt.float32
BF16 = mybir.dt.bfloat16
I16 = mybir.dt.int16
BF = ml_dtypes.bfloat16

LAST_RESULTS = None


def _host_prep(x, edge_index):
    src = np.asarray(edge_index[0], dtype=np.int64)
    tgt = np.asarray(edge_index[1], dtype=np.int64)
    deg = np.bincount(tgt, minlength=N).astype(np.float64) + 1.0
    dinv = (1.0 / np.sqrt(deg)).astype(np.float32)

    # lo set per shard: top (LOP-1) nodes by in-degree
    lo_set = np.zeros(N, bool)
    for c in range(CORES):
        order = np.argsort(-deg[c * SH:(c + 1) * SH], kind="stable")
        lo_set[c * SH + order[:LOP - 1]] = True

    # edges incl self-loops; per-target lo/hi in-counts
    asrc = np.concatenate([src, np.arange(N)])
    atgt = np.concatenate([tgt, np.arange(N)])
    sm = lo_set[asrc]
    locnt = np.bincount(atgt[sm], minlength=N)
    hicnt = np.bincount(atgt[~sm], minlength=N)

    # window-LPT position assignment (per shard, per group)
    WIN = 4
    pos_all = np.empty(N, np.int64)
    for c in range(CORES):
        shard = np.arange(c * SH, (c + 1) * SH)
        lom = lo_set[shard]
        for nodes, npos, nblk, off in ((shard[lom], LOP - 1, LOB, 0),
                                       (shard[~lom], SH - (LOP - 1), HIB, LOP)):
            lo = locnt[nodes]
            ho = hicnt[nodes]
            o = np.argsort(-(lo * 100000 + ho), kind="stable")
            nodes_o = nodes[o]
            hs = ho[o]
            pos_local = np.empty(len(nodes), np.int64)
            i = 0
            b0 = 0
            while i < len(nodes):
                k = min(WIN, nblk - b0)
                caps = np.array([128 if b0 + bb < nblk - 1
                                 else npos - 128 * (nblk - 1)
                                 for bb in range(k)], np.int64)
                wn = min(int(caps.sum()), len(nodes) - i)
                idxw = np.arange(i, i + wn)
                ow = idxw[np.argsort(-hs[idxw], kind="stable")]
                bh = np.zeros(k)
                cnt = np.zeros(k, np.int64)
                for j in ow:
                    best, bestc = -1, None
                    for bb in range(k):
                        if cnt[bb] >= caps[bb]:
                            continue
                        cv = (max(bh[bb], hs[j]) - bh[bb], cnt[bb])
                        if bestc is None or cv < bestc:
                            bestc, best = cv, bb
                    pos_local[j] = off + (b0 + best) * 128 + cnt[best]
                    bh[best] = max(bh[best], hs[j])
                    cnt[best] += 1
                i += wn
                b0 += k
            pos_all[nodes_o] = pos_local

    node_core = np.arange(N) // SH
    # lo table rows are chunk-major (2 chunks of 16 blocks) so the two
    # AllGather chunks write contiguous row ranges
    lo_idx = ((pos_all // LCH) * (CORES * LCH) + node_core * LCH
              + pos_all % LCH)                        # valid where lo_set
    hi_idx = node_core * HIP + (pos_all - LOP)        # valid where ~lo_set

    # per-core per-region CSR -> grids [NBLK, 128, Dmax]
    grids = {}
    cnts = {"lo": np.zeros((CORES, SHP), np.int64),
            "hi": np.zeros((CORES, SHP), np.int64)}
    for c in range(CORES):
        sel = (atgt // SH) == c
        s_c, t_c = asrc[sel], atgt[sel]
        tp = pos_all[t_c]
        mm = lo_set[s_c]
        for reg, m in (("lo", mm), ("hi", ~mm)):
            tpr = tp[m]
            iv = (lo_idx if reg == "lo" else hi_idx)[s_c[m]]
            o = np.argsort(tpr, kind="stable")
            tpr, iv = tpr[o], iv[o]
            cnt = np.bincount(tpr, minlength=SHP)
            cnts[reg][c] = cnt
            starts = np.zeros(SHP + 1, np.int64)
            np.cumsum(cnt, out=starts[1:])
            col = np.arange(len(tpr)) - starts[tpr]
            dmax = int(cnt.max()) if len(tpr) else 1
            pad = LO_ZERO if reg == "lo" else HI_ZERO
            g = np.full((SHP, max(dmax, 1)), pad, np.int64)
            g[tpr, col] = iv
            grids[(c, reg)] = g.reshape(NBLK, 128, -1)

    D_lo = cnts["lo"].reshape(CORES, NBLK, 128).max(axis=(0, 2)).astype(int)
    D_hi = cnts["hi"].reshape(CORES, NBLK, 128).max(axis=(0, 2)).astype(int)

    # block groups of 3 (wide-tile granularity)
    groups = [list(range(b, min(b + 3, NBLK))) for b in range(0, NBLK, 3)]

    # flat slot-major int16 index lists per (core, group, region)
    def flat_idx(c, reg, bs):
        D = D_lo if reg == "lo" else D_hi
        pad = LO_ZERO if reg == "lo" else HI_ZERO
        g = grids[(c, reg)]
        cols = sum(int(D[b]) for b in bs)
        if cols == 0:
            return None
        fl = np.full((cols, 128), pad, np.int64)
        ptr = 0
        for b in bs:
            d = int(D[b])
            w = min(d, g.shape[2])
            fl[ptr:ptr + w, :] = g[b, :, :w].T
            ptr += d
        v = fl.reshape(-1)
        n = cols * 128
        w16 = np.zeros((16, n // 16), np.int16)
        w16[np.arange(n) % 16, np.arange(n) // 16] = v.astype(np.int16)
        # replicate the 16-partition wrap to all 128 partitions (Q7 SIMD read)
        return np.tile(w16, (8, 1))

    idx_arrays = []
    xTs, dinv_blks = [], []
    for c in range(CORES):
        m = {}
        for gi, bs in enumerate(groups):
            a = flat_idx(c, "lo", bs)
            if a is not None:
                m[f"ixl{gi}"] = a
            a = flat_idx(c, "hi", bs)
            if a is not None:
                m[f"ixh{gi}"] = a
        idx_arrays.append(m)

        shard = np.arange(c * SH, (c + 1) * SH)
        pos = pos_all[shard]
        db = np.zeros(SHP, np.float32)
        db[pos] = dinv[shard]
        dinv_blks.append(np.ascontiguousarray(db.reshape(NBLK, 128).T))
        xs = np.zeros((SHP, IN_F), np.float32)
        xs[pos] = np.asarray(x[shard], np.float32)
        xTs.append(np.ascontiguousarray(xs.T).astype(BF))

    return (pos_all, groups, [int(d) for d in D_lo], [int(d) for d in D_hi],
            idx_arrays, xTs, dinv_blks)


def _build_bass(groups, D_lo, D_hi, idx_shapes):
    nc = bacc.Bacc("TRN2", target_bir_lowering=False, debug=False,
                   num_devices=CORES, num_swdge_queues=4)

    xT_in = nc.declare_dram_parameter("xT", [IN_F, SHP], BF16, isOutput=False)
    dinv_in = nc.declare_dram_parameter("dinv_blk", [128, NBLK], F32,
                                        isOutput=False)
    w_in = {}
    for name, shp, dt in [("W1", [IN_F, HP], BF16), ("W2", [H, HP], BF16),
                          ("W3", [H, HP], BF16), ("WL", [H, C_OUT], BF16),
                          ("B1t", [128, H], F32), ("B2t", [128, H], F32),
                          ("B3t", [128, H], F32), ("BLt", [128, C_OUT], F32)]:
        w_in[name] = nc.declare_dram_parameter(name, shp, dt, isOutput=False)
    ix_in = {k: nc.declare_dram_parameter(k, list(shp), I16, isOutput=False)
             for k, shp in idx_shapes.items()}
    out_dram = nc.declare_dram_parameter("out_s", [SHP, C_OUT], F32,
                                         isOutput=True)

    u_sl = [[nc.dram_tensor(f"u_sl{l}_{k}", [LCH, HP], BF16)
             for k in range(2)] for l in range(3)]
    u_sh = [nc.dram_tensor(f"u_sh{l}", [HIP, HP], BF16) for l in range(3)]
    ut_l = [nc.dram_tensor(f"ut_l{l}", [LO_TBL, HP], BF16) for l in range(3)]
    ut_h = [nc.dram_tensor(f"ut_h{l}", [HI_TBL, HP], BF16) for l in range(3)]

    W_next = {0: "W2", 1: "W3"}
    B_of = {0: "B1t", 1: "B2t", 2: "B3t"}
    Lrelu = mybir.ActivationFunctionType.Lrelu

    with tile.TileContext(nc) as tc, ExitStack() as ctx:
        const = ctx.enter_context(tc.tile_pool(name="const", bufs=1))
        wlop = ctx.enter_context(tc.tile_pool(name="wlop", bufs=3))
        whip = ctx.enter_context(tc.tile_pool(name="whip", bufs=3))
        accp = ctx.enter_context(tc.tile_pool(name="accp", bufs=3))
        work = ctx.enter_context(tc.tile_pool(name="work", bufs=4))
        upool = ctx.enter_context(tc.tile_pool(name="upool", bufs=4))
        opool = ctx.enter_context(tc.tile_pool(name="opool", bufs=3))
        psum = ctx.enter_context(tc.tile_pool(name="psum", bufs=2,
                                              space="PSUM"))

        # ---- constants ----
        xT = const.tile([IN_F, SHP], BF16)
        nc.sync.dma_start(xT[:], xT_in[:])
        dinv = const.tile([128, NBLK], F32)
        nc.sync.dma_start(dinv[:], dinv_in[:])
        wt = {}
        for name in w_in:
            t = const.tile(list(w_in[name].shape), w_in[name].dtype, tag=name)
            nc.sync.dma_start(t[:], w_in[name][:])
            wt[name] = t
        ix = {}
        for k in ix_in:
            t = const.tile(list(ix_in[k].shape), I16, tag=k)
            nc.sync.dma_start(t[:], ix_in[k][:])
            ix[k] = t
        ident = const.tile([128, 128], F32)
        make_identity(nc, ident[:])

        def store_u(l, b, ub):
            if b < LOB:
                k, bb = divmod(b, 16)
                nc.sync.dma_start(
                    u_sl[l][k][bb * 128:(bb + 1) * 128, :], ub[:])
            else:
                bb = b - LOB
                nc.sync.dma_start(u_sh[l][bb * 128:(bb + 1) * 128, :], ub[:])

        # ---- layer-1 table: u0 = dinv * (x @ W1) ----
        for b in range(NBLK):
            vP = psum.tile([128, HP], F32, tag="vP")
            nc.tensor.matmul(vP[:], lhsT=xT[:, b * 128:(b + 1) * 128],
                             rhs=wt["W1"][:], start=True, stop=True)
            ub = upool.tile([128, HP], BF16, tag="ub")
            nc.vector.tensor_scalar(ub[:], vP[:], dinv[:, b:b + 1], None,
                                    op0=mybir.AluOpType.mult)
            store_u(0, b, ub)

        qrr = [0]

        def gather_windows(wide, table, ixt, cols):
            """Fill wide [128, cols, 128] via <=GMAX-column dma_gather calls,
            round-robined over the 4 SWDGE queues (4 Q7 pairs + rings run
            descriptor generation and drain in parallel)."""
            a = 0
            while a < cols:
                b = min(a + GMAX, cols)
                n = (b - a) * 128
                nc.gpsimd.dma_gather(
                    wide[:, a:b, :], table[:], ixt[:, a * 8:b * 8], n, n, HP,
                    transpose=False, queue_num=qrr[0] % 4)
                qrr[0] += 1
                a = b

        def first_level(dst3, src3, n):
            """dst fp32 [128, ceil(n/2), H] = pairwise sums of src bf16 cols."""
            h = n - n // 2
            n2 = n // 2
            if n2:
                nc.vector.tensor_tensor(out=dst3[:, 0:n2, :],
                                        in0=src3[:, 0:n2, :H],
                                        in1=src3[:, h:h + n2, :H],
                                        op=mybir.AluOpType.add)
            if h > n2:
                nc.vector.tensor_copy(dst3[:, n2:h, :], src3[:, n2:h, :H])
            return h

        # ---- layers ----
        for l in range(3):
            for k in range(2):
                nc.gpsimd.collective_compute(
                    "AllGather", mybir.AluOpType.bypass,
                    replica_groups=[list(range(CORES))],
                    ins=[u_sl[l][k][:]],
                    outs=[ut_l[l][k * CORES * LCH:(k + 1) * CORES * LCH, :]])
            nc.gpsimd.collective_compute(
                "AllGather", mybir.AluOpType.bypass,
                replica_groups=[list(range(CORES))],
                ins=[u_sh[l][:]], outs=[ut_h[l][:]])

            for gi, bs in enumerate(groups):
                CL = sum(D_lo[b] for b in bs)
                CH = sum(D_hi[b] for b in bs)
                wlo = whi = None
                if CL:
                    wlo = wlop.tile([128, CL, HP], BF16, tag="wlo")
                    gather_windows(wlo, ut_l[l], ix[f"ixl{gi}"], CL)
                if CH:
                    whi = whip.tile([128, CH, HP], BF16, tag="whi")
                    gather_windows(whi, ut_h[l], ix[f"ixh{gi}"], CH)

                olo = ohi = 0
                for b in bs:
                    nlo, nhi = D_lo[b], D_hi[b]
                    hlo = nlo - nlo // 2
                    hhi = nhi - nhi // 2
                    acc = accp.tile([128, hlo + hhi, H], F32, tag="acc")
                    if nlo:
                        first_level(acc[:, 0:hlo, :],
                                    wlo[:, olo:olo + nlo, :], nlo)
                    if nhi:
                        first_level(acc[:, hlo:hlo + hhi, :],
                                    whi[:, ohi:ohi + nhi, :], nhi)
                    olo += nlo
                    ohi += nhi
                    cnt = hlo + hhi
                    while cnt > 1:
                        hh = cnt // 2
                        nc.vector.tensor_tensor(
                            out=acc[:, 0:hh, :], in0=acc[:, 0:hh, :],
                            in1=acc[:, cnt - hh:cnt, :],
                            op=mybir.AluOpType.add)
                        cnt -= hh
                    # h = Lrelu(dinv_t * s + b), row-major [128, H]
                    t1 = work.tile([128, H], F32, tag="t1")
                    nc.vector.tensor_scalar(t1[:], acc[:, 0, :],
                                            dinv[:, b:b + 1], None,
                                            op0=mybir.AluOpType.mult)
                    t2 = work.tile([128, H], F32, tag="t2")
                    nc.vector.tensor_tensor(t2[:], t1[:], wt[B_of[l]][:],
                                            op=mybir.AluOpType.add)
                    h = work.tile([128, H], F32, tag="h")
                    nc.scalar.activation(h[:], t2[:], Lrelu, bias=0.0,
                                         scale=1.0, alpha=SLOPE)
                    trP = psum.tile([H, 128], F32, tag="trP")
                    nc.tensor.transpose(trP[:], h[:], ident[:])
                    hsT = work.tile([H, 128], BF16, tag="hsT")
                    nc.scalar.copy(hsT[:], trP[:])
                    if l < 2:
                        vP = psum.tile([128, HP], F32, tag="vP")
                        nc.tensor.matmul(vP[:], lhsT=hsT[:],
                                         rhs=wt[W_next[l]][:],
                                         start=True, stop=True)
                        ub = upool.tile([128, HP], BF16, tag="ub")
                        nc.vector.tensor_scalar(ub[:], vP[:],
                                                dinv[:, b:b + 1], None,
                                                op0=mybir.AluOpType.mult)
                        store_u(l + 1, b, ub)
                    else:
                        oP = psum.tile([128, C_OUT], F32, tag="oP")
                        nc.tensor.matmul(oP[:], lhsT=hsT[:],
                                         rhs=wt["WL"][:],
                                         start=True, stop=True)
                        o = opool.tile([128, C_OUT], F32, tag="o")
                        nc.vector.tensor_tensor(o[:], oP[:], wt["BLt"][:],
                                                op=mybir.AluOpType.add)
                        nc.sync.dma_start(out_dram[b * 128:(b + 1) * 128, :],
                                          o[:])
    nc.compile()
    return nc


def _ensure_ntff_hook():
    """The agent image's antenv lacks axon_hooks; shim it and register the
    ctypes NTFF profiling hook so trace=True works under axon."""
    import sys as _sys
    import types
    try:
        import antenv.axon_hooks  # noqa: F401
        return
    except ImportError:
        pass
    mod = types.ModuleType("antenv.axon_hooks")
    _h = [None]
    mod.set_axon_ntff_profile_hook = lambda hook: _h.__setitem__(0, hook)
    mod.get_axon_ntff_profile_hook = lambda: _h[0]
    _sys.modules["antenv.axon_hooks"] = mod
    try:
        from trn_agent_boot.trn_boot import _ntff_profile_via_ctypes
        hook = _ntff_profile_via_ctypes("/opt/axon/libaxon_pjrt.so")
        if hook is not None:
            mod.set_axon_ntff_profile_hook(hook)
    except Exception:
        pass


def kernel(x, edge_index, W1, b1, W2, b2, W3, b3, Wl, bl):
    global LAST_RESULTS
    x = np.asarray(x, dtype=np.float32)
    (pos_all, groups, D_lo, D_hi, idx_arrays, xTs,
     dinv_blks) = _host_prep(x, edge_index)

    idx_shapes = {k: v.shape for k, v in idx_arrays[0].items()}
    nc = _build_bass(groups, D_lo, D_hi, idx_shapes)

    def padw(w, cols):
        out = np.zeros((w.shape[0], cols), np.float32)
        out[:, :w.shape[1]] = w
        return out.astype(BF)

    shared = {
        "W1": padw(np.asarray(W1, np.float32), HP),
        "W2": padw(np.asarray(W2, np.float32), HP),
        "W3": padw(np.asarray(W3, np.float32), HP),
        "WL": np.asarray(Wl, np.float32).astype(BF),
        "B1t": np.tile(np.asarray(b1, np.float32), (128, 1)),
        "B2t": np.tile(np.asarray(b2, np.float32), (128, 1)),
        "B3t": np.tile(np.asarray(b3, np.float32), (128, 1)),
        "BLt": np.tile(np.asarray(bl, np.float32), (128, 1)),
    }
    in_maps = []
    for c in range(CORES):
        m = dict(shared)
        m["xT"] = xTs[c]
        m["dinv_blk"] = dinv_blks[c]
        m.update(idx_arrays[c])
        in_maps.append(m)

    trace = bool(int(os.environ.get("GCN_TRACE", "0")))
    if trace:
        _ensure_ntff_hook()
    res = run_bass_kernel_spmd(nc, in_maps, list(range(CORES)), trace=trace)
    LAST_RESULTS = res

    out = np.empty((N, C_OUT), dtype=np.float32)
    for c in range(CORES):
        shard = res.results[c]["out_s"]
        out[c * SH:(c + 1) * SH] = shard[pos_all[c * SH:(c + 1) * SH]]
    return out


# revision 9
# speedup vs baseline: 1.0488x; 1.0488x over previous
"""3-layer GCN on 8 Trainium2 NeuronCores (Bass/Tile), batched-gather version.

Math: with A = D^-1/2 (Adj + I) D^-1/2 (PyG GCNConv norm, self-loops),
each layer is h' = leaky_relu(A h W + b). Factor A h = dinv * ((Adj+I)(dinv*h)),
so aggregation is an unweighted gather-sum over in-edges (self-loop folded in
as an ordinary edge) of the row-scaled table u = dinv*(h@W).

Device strategy (per core, nodes sharded 6250/core):
- The per-layer table u is bf16 [rows, 128] (feature dim padded 96->128 so a
  row is 256 B, the minimum dma_gather elem size). Node positions inside each
  shard are split into a "lo" group (4096 positions, blocks 0..31) and a "hi"
  group (2176 positions, blocks 32..48) so each AllGather'd table region
  (8*4096=32768 / 8*2176=17408 rows) stays within the signed-int16 index range
  of the Q7 dma_gather ucode.
- Per layer: two AllGathers (lo region first) replicate the table, then
  dma_gather instructions (non-transpose -> row-major [128, cols, 128] wide
  tiles; idx i lands at [i%128, i//128, :]) pull all neighbor rows using
  host-built slot-major index lists. Each instruction moves <=1024 rows (the
  SWDGE descriptor-ring capacity - found empirically; bigger wedges the
  device) but that still amortizes the ~1us per-instruction SWDGE overhead
  ~8x better than per-column indirect DMA.
- Per target block: pairwise tree-sum (bf16 -> fp32) over gathered columns
  ([:, :, :96] slices), scale by dinv[target], add bias, Lrelu on the scalar
  engine, PE-transpose, then the next layer's matmul.
- Padding: position 4095 and positions 6251..6271 of each shard are reserved
  zero rows (dinv=0 there forces u rows to 0); grid padding points at them.
- Host-side: targets are assigned to blocks by a window-LPT heuristic that
  first equalizes per-block lo-degree, then balances hi-degree inside small
  windows, minimizing gather padding (~1.18x over the dense lower bound).
- Index lists are int16, wrapped [16, n/16] (idx i at [i%16, i//16]) and
  replicated to all 128 partitions (the Q7 pairs read (queue+1)*2*16 SIMD
  channels).
"""
import os
import numpy as np
import ml_dtypes
from contextlib import ExitStack

import concourse.bass as bass
import concourse.tile as tile
from concourse import bacc, mybir
from concourse.bass_utils import run_bass_kernel_spmd
from concourse.masks import make_identity

N = 50000
E = 800000
IN_F = 128
H = 96
HP = 128                  # padded feature dim (table row = 256B bf16)
C_OUT = 21
CORES = 8
SH = N // CORES           # 6250 real nodes per core
NBLK = 49
SHP = NBLK * 128          # 6272 padded positions per shard
LOB = 32                  # lo blocks per shard
HIB = NBLK - LOB          # 17 hi blocks
LOP = LOB * 128           # 4096 lo positions (4095 real + 1 zero)
HIP = HIB * 128           # 2176 hi positions (2155 real + 21 zero)
LO_TBL = CORES * LOP      # 32768 (= int16 max + 1: just fits)
HI_TBL = CORES * HIP      # 17408
LCH = LOP // 2            # lo AllGather chunk: 2048 positions (16 blocks)
LO_ZERO = CORES * LCH + 2047  # zero row (pos 4095, shard 0) in lo table
HI_ZERO = SH - (LOP - 1)  # 2155: shard-0 zero row in hi table
GMAX = 8                  # max 8 columns (1024 idxs) per dma_gather
SLOPE = 0.01

F32 = mybir.dt.float32
BF16 = mybir.dt.bfloat16
I16 = mybir.dt.int16
BF = ml_dtypes.bfloat16

LAST_RESULTS = None


def _host_prep(x, edge_index):
    src = np.asarray(edge_index[0], dtype=np.int64)
    tgt = np.asarray(edge_index[1], dtype=np.int64)
    deg = np.bincount(tgt, minlength=N).astype(np.float64) + 1.0
    dinv = (1.0 / np.sqrt(deg)).astype(np.float32)

    # lo set per shard: top (LOP-1) nodes by in-degree
    lo_set = np.zeros(N, bool)
    for c in range(CORES):
        order = np.argsort(-deg[c * SH:(c + 1) * SH], kind="stable")
        lo_set[c * SH + order[:LOP - 1]] = True

    # edges incl self-loops; per-target lo/hi in-counts
    asrc = np.concatenate([src, np.arange(N)])
    atgt = np.concatenate([tgt, np.arange(N)])
    sm = lo_set[asrc]
    locnt = np.bincount(atgt[sm], minlength=N)
    hicnt = np.bincount(atgt[~sm], minlength=N)

    # window-LPT position assignment (per shard, per group)
    WIN = 4
    pos_all = np.empty(N, np.int64)
    for c in range(CORES):
        shard = np.arange(c * SH, (c + 1) * SH)
        lom = lo_set[shard]
        for nodes, npos, nblk, off in ((shard[lom], LOP - 1, LOB, 0),
                                       (shard[~lom], SH - (LOP - 1), HIB, LOP)):
            lo = locnt[nodes]
            ho = hicnt[nodes]
            o = np.argsort(-(lo * 100000 + ho), kind="stable")
            nodes_o = nodes[o]
            hs = ho[o]
            pos_local = np.empty(len(nodes), np.int64)
            i = 0
            b0 = 0
            while i < len(nodes):
                k = min(WIN, nblk - b0)
                caps = np.array([128 if b0 + bb < nblk - 1
                                 else npos - 128 * (nblk - 1)
                                 for bb in range(k)], np.int64)
                wn = min(int(caps.sum()), len(nodes) - i)
                idxw = np.arange(i, i + wn)
                ow = idxw[np.argsort(-hs[idxw], kind="stable")]
                bh = np.zeros(k)
                cnt = np.zeros(k, np.int64)
                for j in ow:
                    best, bestc = -1, None
                    for bb in range(k):
                        if cnt[bb] >= caps[bb]:
                            continue
                        cv = (max(bh[bb], hs[j]) - bh[bb], cnt[bb])
                        if bestc is None or cv < bestc:
                            bestc, best = cv, bb
                    pos_local[j] = off + (b0 + best) * 128 + cnt[best]
                    bh[best] = max(bh[best], hs[j])
                    cnt[best] += 1
                i += wn
                b0 += k
            pos_all[nodes_o] = pos_local

    node_core = np.arange(N) // SH
    # lo table rows are chunk-major (2 chunks of 16 blocks) so the two
    # AllGather chunks write contiguous row ranges
    lo_idx = ((pos_all // LCH) * (CORES * LCH) + node_core * LCH
              + pos_all % LCH)                        # valid where lo_set
    hi_idx = node_core * HIP + (pos_all - LOP)        # valid where ~lo_set

    # per-core per-region CSR -> grids [NBLK, 128, Dmax]
    grids = {}
    cnts = {"lo": np.zeros((CORES, SHP), np.int64),
            "hi": np.zeros((CORES, SHP), np.int64)}
    for c in range(CORES):
        sel = (atgt // SH) == c
        s_c, t_c = asrc[sel], atgt[sel]
        tp = pos_all[t_c]
        mm = lo_set[s_c]
        for reg, m in (("lo", mm), ("hi", ~mm)):
            tpr = tp[m]
            iv = (lo_idx if reg == "lo" else hi_idx)[s_c[m]]
            o = np.argsort(tpr, kind="stable")
            tpr, iv = tpr[o], iv[o]
            cnt = np.bincount(tpr, minlength=SHP)
            cnts[reg][c] = cnt
            starts = np.zeros(SHP + 1, np.int64)
            np.cumsum(cnt, out=starts[1:])
            col = np.arange(len(tpr)) - starts[tpr]
            dmax = int(cnt.max()) if len(tpr) else 1
            pad = LO_ZERO if reg == "lo" else HI_ZERO
            g = np.full((SHP, max(dmax, 1)), pad, np.int64)
            g[tpr, col] = iv
            grids[(c, reg)] = g.reshape(NBLK, 128, -1)

    D_lo = cnts["lo"].reshape(CORES, NBLK, 128).max(axis=(0, 2)).astype(int)
    D_hi = cnts["hi"].reshape(CORES, NBLK, 128).max(axis=(0, 2)).astype(int)

    # block groups of 3 (wide-tile granularity)
    groups = [list(range(b, min(b + 3, NBLK))) for b in range(0, NBLK, 3)]

    # flat slot-major int16 index lists per (core, group, region)
    def flat_idx(c, reg, bs):
        D = D_lo if reg == "lo" else D_hi
        pad = LO_ZERO if reg == "lo" else HI_ZERO
        g = grids[(c, reg)]
        cols = sum(int(D[b]) for b in bs)
        if cols == 0:
            return None
        fl = np.full((cols, 128), pad, np.int64)
        ptr = 0
        for b in bs:
            d = int(D[b])
            w = min(d, g.shape[2])
            fl[ptr:ptr + w, :] = g[b, :, :w].T
            ptr += d
        v = fl.reshape(-1)
        n = cols * 128
        w16 = np.zeros((16, n // 16), np.int16)
        w16[np.arange(n) % 16, np.arange(n) // 16] = v.astype(np.int16)
        # replicate the 16-partition wrap to all 128 partitions (Q7 SIMD read)
        return np.tile(w16, (8, 1))

    idx_arrays = []
    xTs, dinv_blks = [], []
    for c in range(CORES):
        m = {}
        for gi, bs in enumerate(groups):
            a = flat_idx(c, "lo", bs)
            if a is not None:
                m[f"ixl{gi}"] = a
            a = flat_idx(c, "hi", bs)
            if a is not None:
                m[f"ixh{gi}"] = a
        idx_arrays.append(m)

        shard = np.arange(c * SH, (c + 1) * SH)
        pos = pos_all[shard]
        db = np.zeros(SHP, np.float32)
        db[pos] = dinv[shard]
        dinv_blks.append(np.ascontiguousarray(db.reshape(NBLK, 128).T))
        xs = np.zeros((SHP, IN_F), np.float32)
        xs[pos] = np.asarray(x[shard], np.float32)
        xTs.append(np.ascontiguousarray(xs.T).astype(BF))

    return (pos_all, groups, [int(d) for d in D_lo], [int(d) for d in D_hi],
            idx_arrays, xTs, dinv_blks)


def _build_bass(groups, D_lo, D_hi, idx_shapes):
    nc = bacc.Bacc("TRN2", target_bir_lowering=False, debug=False,
                   num_devices=CORES, num_swdge_queues=4)

    xT_in = nc.declare_dram_parameter("xT", [IN_F, SHP], BF16, isOutput=False)
    dinv_in = nc.declare_dram_parameter("dinv_blk", [128, NBLK], F32,
                                        isOutput=False)
    w_in = {}
    for name, shp, dt in [("W1", [IN_F, HP], BF16), ("W2", [H, HP], BF16),
                          ("W3", [H, HP], BF16), ("WL", [H, C_OUT], BF16),
                          ("B1t", [128, H], F32), ("B2t", [128, H], F32),
                          ("B3t", [128, H], F32), ("BLt", [128, C_OUT], F32)]:
        w_in[name] = nc.declare_dram_parameter(name, shp, dt, isOutput=False)
    ix_in = {k: nc.declare_dram_parameter(k, list(shp), I16, isOutput=False)
             for k, shp in idx_shapes.items()}
    out_dram = nc.declare_dram_parameter("out_s", [SHP, C_OUT], F32,
                                         isOutput=True)

    u_sl = [[nc.dram_tensor(f"u_sl{l}_{k}", [LCH, HP], BF16)
             for k in range(2)] for l in range(3)]
    u_sh = [nc.dram_tensor(f"u_sh{l}", [HIP, HP], BF16) for l in range(3)]
    ut_l = [nc.dram_tensor(f"ut_l{l}", [LO_TBL, HP], BF16) for l in range(3)]
    ut_h = [nc.dram_tensor(f"ut_h{l}", [HI_TBL, HP], BF16) for l in range(3)]

    W_next = {0: "W2", 1: "W3"}
    B_of = {0: "B1t", 1: "B2t", 2: "B3t"}
    Lrelu = mybir.ActivationFunctionType.Lrelu

    with tile.TileContext(nc) as tc, ExitStack() as ctx:
        const = ctx.enter_context(tc.tile_pool(name="const", bufs=1))
        wlop = ctx.enter_context(tc.tile_pool(name="wlop", bufs=4))
        whip = ctx.enter_context(tc.tile_pool(name="whip", bufs=4))
        work = ctx.enter_context(tc.tile_pool(name="work", bufs=4))
        upool = ctx.enter_context(tc.tile_pool(name="upool", bufs=4))
        opool = ctx.enter_context(tc.tile_pool(name="opool", bufs=3))
        psum = ctx.enter_context(tc.tile_pool(name="psum", bufs=2,
                                              space="PSUM"))

        # ---- constants ----
        xT = const.tile([IN_F, SHP], BF16)
        nc.sync.dma_start(xT[:], xT_in[:])
        dinv = const.tile([128, NBLK], F32)
        nc.sync.dma_start(dinv[:], dinv_in[:])
        wt = {}
        for name in w_in:
            t = const.tile(list(w_in[name].shape), w_in[name].dtype, tag=name)
            nc.sync.dma_start(t[:], w_in[name][:])
            wt[name] = t
        ix = {}
        for k in ix_in:
            t = const.tile(list(ix_in[k].shape), I16, tag=k)
            nc.sync.dma_start(t[:], ix_in[k][:])
            ix[k] = t
        ident = const.tile([128, 128], F32)
        make_identity(nc, ident[:])

        def store_u(l, b, ub):
            if b < LOB:
                k, bb = divmod(b, 16)
                nc.sync.dma_start(
                    u_sl[l][k][bb * 128:(bb + 1) * 128, :], ub[:])
            else:
                bb = b - LOB
                nc.sync.dma_start(u_sh[l][bb * 128:(bb + 1) * 128, :], ub[:])

        # ---- layer-1 table: u0 = dinv * (x @ W1) ----
        for b in range(NBLK):
            vP = psum.tile([128, HP], F32, tag="vP")
            nc.tensor.matmul(vP[:], lhsT=xT[:, b * 128:(b + 1) * 128],
                             rhs=wt["W1"][:], start=True, stop=True)
            ub = upool.tile([128, HP], BF16, tag="ub")
            nc.vector.tensor_scalar(ub[:], vP[:], dinv[:, b:b + 1], None,
                                    op0=mybir.AluOpType.mult)
            store_u(0, b, ub)

        qrr = [0]

        def gather_windows(wide, table, ixt, cols):
            """Fill wide [128, cols, 128] via <=GMAX-column dma_gather calls,
            round-robined over the 4 SWDGE queues (4 Q7 pairs + rings run
            descriptor generation and drain in parallel)."""
            a = 0
            while a < cols:
                b = min(a + GMAX, cols)
                n = (b - a) * 128
                nc.gpsimd.dma_gather(
                    wide[:, a:b, :], table[:], ixt[:, a * 8:b * 8], n, n, HP,
                    transpose=False, queue_num=qrr[0] % 4)
                qrr[0] += 1
                a = b

        def tree_sum(w3, off, n):
            """In-place bf16 pairwise halving over w3[:, off:off+n, :H]."""
            cnt = n
            while cnt > 1:
                hh = cnt // 2
                nc.vector.tensor_tensor(
                    out=w3[:, off:off + hh, :H],
                    in0=w3[:, off:off + hh, :H],
                    in1=w3[:, off + cnt - hh:off + cnt, :H],
                    op=mybir.AluOpType.add)
                cnt -= hh

        # ---- layers ----
        for l in range(3):
            for k in range(2):
                nc.gpsimd.collective_compute(
                    "AllGather", mybir.AluOpType.bypass,
                    replica_groups=[list(range(CORES))],
                    ins=[u_sl[l][k][:]],
                    outs=[ut_l[l][k * CORES * LCH:(k + 1) * CORES * LCH, :]])
            nc.gpsimd.collective_compute(
                "AllGather", mybir.AluOpType.bypass,
                replica_groups=[list(range(CORES))],
                ins=[u_sh[l][:]], outs=[ut_h[l][:]])

            for gi, bs in enumerate(groups):
                CL = sum(D_lo[b] for b in bs)
                CH = sum(D_hi[b] for b in bs)
                wlo = whi = None
                if CL:
                    wlo = wlop.tile([128, CL, HP], BF16, tag="wlo")
                    gather_windows(wlo, ut_l[l], ix[f"ixl{gi}"], CL)
                if CH:
                    whi = whip.tile([128, CH, HP], BF16, tag="whi")
                    gather_windows(whi, ut_h[l], ix[f"ixh{gi}"], CH)

                olo = ohi = 0
                for b in bs:
                    nlo, nhi = D_lo[b], D_hi[b]
                    if nlo:
                        tree_sum(wlo, olo, nlo)
                    if nhi:
                        tree_sum(whi, ohi, nhi)
                    s = work.tile([128, H], F32, tag="s")
                    if nlo and nhi:
                        nc.vector.tensor_tensor(s[:], wlo[:, olo, :H],
                                                whi[:, ohi, :H],
                                                op=mybir.AluOpType.add)
                    elif nlo:
                        nc.vector.tensor_copy(s[:], wlo[:, olo, :H])
                    else:
                        nc.vector.tensor_copy(s[:], whi[:, ohi, :H])
                    olo += nlo
                    ohi += nhi
                    # h = Lrelu(dinv_t * s + b), row-major [128, H]
                    t1 = work.tile([128, H], F32, tag="t1")
                    nc.vector.tensor_scalar(t1[:], s[:],
                                            dinv[:, b:b + 1], None,
                                            op0=mybir.AluOpType.mult)
                    t2 = work.tile([128, H], F32, tag="t2")
                    nc.vector.tensor_tensor(t2[:], t1[:], wt[B_of[l]][:],
                                            op=mybir.AluOpType.add)
                    h = work.tile([128, H], F32, tag="h")
                    nc.scalar.activation(h[:], t2[:], Lrelu, bias=0.0,
                                         scale=1.0, alpha=SLOPE)
                    trP = psum.tile([H, 128], F32, tag="trP")
                    nc.tensor.transpose(trP[:], h[:], ident[:])
                    hsT = work.tile([H, 128], BF16, tag="hsT")
                    nc.scalar.copy(hsT[:], trP[:])
                    if l < 2:
                        vP = psum.tile([128, HP], F32, tag="vP")
                        nc.tensor.matmul(vP[:], lhsT=hsT[:],
                                         rhs=wt[W_next[l]][:],
                                         start=True, stop=True)
                        ub = upool.tile([128, HP], BF16, tag="ub")
                        nc.vector.tensor_scalar(ub[:], vP[:],
                                                dinv[:, b:b + 1], None,
                                                op0=mybir.AluOpType.mult)
                        store_u(l + 1, b, ub)
                    else:
                        oP = psum.tile([128, C_OUT], F32, tag="oP")
                        nc.tensor.matmul(oP[:], lhsT=hsT[:],
                                         rhs=wt["WL"][:],
                                         start=True, stop=True)
                        o = opool.tile([128, C_OUT], F32, tag="o")
                        nc.vector.tensor_tensor(o[:], oP[:], wt["BLt"][:],
                                                op=mybir.AluOpType.add)
                        nc.sync.dma_start(out_dram[b * 128:(b + 1) * 128, :],
                                          o[:])
    nc.compile()
    return nc


def _ensure_ntff_hook():
    """The agent image's antenv lacks axon_hooks; shim it and register the
    ctypes NTFF profiling hook so trace=True works under axon."""
    import sys as _sys
    import types
    try:
        import antenv.axon_hooks  # noqa: F401
        return
    except ImportError:
        pass
    mod = types.ModuleType("antenv.axon_hooks")
    _h = [None]
    mod.set_axon_ntff_profile_hook = lambda hook: _h.__setitem__(0, hook)
    mod.get_axon_ntff_profile_hook = lambda: _h[0]
    _sys.modules["antenv.axon_hooks"] = mod
    try:
        from trn_agent_boot.trn_boot import _ntff_profile_via_ctypes
        hook = _ntff_profile_via_ctypes("/opt/axon/libaxon_pjrt.so")
        if hook is not None:
            mod.set_axon_ntff_profile_hook(hook)
    except Exception:
        pass


def kernel(x, edge_index, W1, b1, W2, b2, W3, b3, Wl, bl):
    global LAST_RESULTS
    x = np.asarray(x, dtype=np.float32)
    (pos_all, groups, D_lo, D_hi, idx_arrays, xTs,
     dinv_blks) = _host_prep(x, edge_index)

    idx_shapes = {k: v.shape for k, v in idx_arrays[0].items()}
    nc = _build_bass(groups, D_lo, D_hi, idx_shapes)

    def padw(w, cols):
        out = np.zeros((w.shape[0], cols), np.float32)
        out[:, :w.shape[1]] = w
        return out.astype(BF)

    shared = {
        "W1": padw(np.asarray(W1, np.float32), HP),
        "W2": padw(np.asarray(W2, np.float32), HP),
        "W3": padw(np.asarray(W3, np.float32), HP),
        "WL": np.asarray(Wl, np.float32).astype(BF),
        "B1t": np.tile(np.asarray(b1, np.float32), (128, 1)),
        "B2t": np.tile(np.asarray(b2, np.float32), (128, 1)),
        "B3t": np.tile(np.asarray(b3, np.float32), (128, 1)),
        "BLt": np.tile(np.asarray(bl, np.float32), (128, 1)),
    }
    in_maps = []
    for c in range(CORES):
        m = dict(shared)
        m["xT"] = xTs[c]
        m["dinv_blk"] = dinv_blks[c]
        m.update(idx_arrays[c])
        in_maps.append(m)

    trace = bool(int(os.environ.get("GCN_TRACE", "0")))
    if trace:
        _ensure_ntff_hook()
    res = run_bass_kernel_spmd(nc, in_maps, list(range(CORES)), trace=trace)
    LAST_RESULTS = res

    out = np.empty((N, C_OUT), dtype=np.float32)
    for c in range(CORES):
        shard = res.results[c]["out_s"]
        out[c * SH:(c + 1) * SH] = shard[pos_all[c * SH:(c + 1) * SH]]
    return out
